# revision 1
# baseline (speedup 1.0000x reference)
"""BiaffineAttn Trainium2 kernel.

Math (per batch b):
    t    = x2 @ U                      [S, D]
    attn = t @ x1^T + (x1 @ bias)[None, :]
    p    = softmax(attn, axis=-1)
    out  = relu((p @ x1) @ fc_w^T + fc_b)    [S, F]

Sharding: data-parallel over batch B=8, one batch per NeuronCore.

Per-core pipeline (all matmuls in fp32r = fp22-truncated fp32, which streams
at 1 cycle/row on the PE vs 4 for true fp32; N=512 moving chunks):
  The whole attention block is computed in TRANSPOSED orientation so that the
  softmax key dimension (t') lands on SBUF partitions:
    tT      = (x2 @ U)^T          stationary U[d,e] chunks, moving x2T[d,s]
    scoresT = attn^T [t', s]      stationary x1T[e,t'] 128x128 tiles, moving tT
    pT      = exp(scoresT - rowmax_bcast + kb)   (exact per-row max; kb is the
                                   per-key additive bias folded into exp's
                                   per-partition bias operand)
    oT      = (p @ x1)^T          stationary x1[t',e] chunks, moving pT
    outT    = relu((oT^T @ fcwT)^T * recip + fcb) stationary fcwT[e,f], moving oT
  rowmax: running elementwise max over the 16 t'-tiles of scoresT, then a
  128-partition reduce via 4 PE transposes, reassembled into a broadcast tile
  with K=1 ones-matmuls.  rowsum: ones-column matmuls accumulating over t'.

Host side: transposes x1/x2/fc_w per-core (layout prep for DMA-efficient
loads; fp32 DMA transpose does not exist on TRN2) and transposes the [F,S]
per-core output back to [S,F] when gathering.
"""

import os
import sys
from contextlib import ExitStack

import numpy as np

for _p in ("/opt/trn_rl_repo", os.path.expanduser("~/.axon_site/_ro/trn_rl_repo")):
    if os.path.isdir(_p) and _p not in sys.path:
        sys.path.insert(0, _p)

import concourse.bass as bass
import concourse.mybir as mybir
import concourse.tile as tile
from concourse import bacc

B = 8
S = 2048          # sequence length (both s and t')
D = 1024          # d_model
F = 512           # fc output dim
P = 128
SB = 512          # s superblock (moving free dim of every matmul)
NSB = S // SB     # 4
DC = D // P       # 8 contraction chunks of d / e
TC = S // P       # 16 t' tiles
FP32 = mybir.dt.float32
FP32R = mybir.dt.float32r
BF16 = mybir.dt.bfloat16
AF = mybir.ActivationFunctionType
ALU = mybir.AluOpType
AX = mybir.AxisListType


def build_nc():
    nc = bacc.Bacc(
        "TRN2",
        target_bir_lowering=False,
        debug=False,
        enable_asserts=False,
    )

    x1_d = nc.dram_tensor("x1", [S, D], BF16, kind="ExternalInput")
    x1t_d = nc.dram_tensor("x1t", [D, S], FP32R, kind="ExternalInput")
    x2t_d = nc.dram_tensor("x2t", [D, S], FP32R, kind="ExternalInput")
    u_d = nc.dram_tensor("u", [D, D], FP32R, kind="ExternalInput")
    fcwt_d = nc.dram_tensor("fcwt", [D, F], FP32R, kind="ExternalInput")
    bias_d = nc.dram_tensor("bias", [D, 1], FP32R, kind="ExternalInput")
    fcb_d = nc.dram_tensor("fcb", [F, 1], FP32, kind="ExternalInput")
    outt_d = nc.dram_tensor("outt", [F, S], FP32, kind="ExternalOutput")

    with tile.TileContext(nc) as tc, ExitStack() as ctx:
        # ---------- pools ----------
        p_u = ctx.enter_context(tc.tile_pool(name="ures", bufs=DC))
        p_x1 = ctx.enter_context(tc.tile_pool(name="x1res", bufs=TC))
        p_kb = ctx.enter_context(tc.tile_pool(name="kbcols", bufs=TC))
        p_bc = ctx.enter_context(tc.tile_pool(name="biascols", bufs=DC))
        p_fcb = ctx.enter_context(tc.tile_pool(name="fcbcols", bufs=F // P))
        p_ones = ctx.enter_context(tc.tile_pool(name="ones", bufs=1))
        p_psum = ctx.enter_context(tc.tile_pool(name="psum", bufs=8, space="PSUM"))
        p_x1tc = ctx.enter_context(tc.tile_pool(name="x1tcs", bufs=9))
        p_x2t = ctx.enter_context(tc.tile_pool(name="x2ts", bufs=8))
        p_tt = ctx.enter_context(tc.tile_pool(name="tts", bufs=8))
        p_sc = ctx.enter_context(tc.tile_pool(name="scores", bufs=TC))
        p_pb = ctx.enter_context(tc.tile_pool(name="pbf", bufs=TC))
        p_ot = ctx.enter_context(tc.tile_pool(name="ots", bufs=8))
        p_aux = ctx.enter_context(tc.tile_pool(name="aux", bufs=1))
        p_row = ctx.enter_context(tc.tile_pool(name="rows", bufs=1))
        p_out = ctx.enter_context(tc.tile_pool(name="outs", bufs=1))
        p_fcw = ctx.enter_context(tc.tile_pool(name="fcws", bufs=3))

        # ---------- small/fast inputs first: U, bias cols, identity ----------
        u_tiles = []
        for i in range(DC):
            u_t = p_u.tile([P, D], FP32R, name=f"ur{i}", tag="ur")
            nc.sync.dma_start(u_t[:], u_d[i * P : (i + 1) * P, :])
            u_tiles.append(u_t)
        bias_cols = []
        for i in range(DC):
            b_t = p_bc.tile([P, 1], FP32R, name=f"bc{i}", tag="bc")
            nc.sync.dma_start(b_t[:], bias_d[i * P : (i + 1) * P, :])
            bias_cols.append(b_t)
        fcb_cols = []
        for i in range(F // P):
            c_t = p_fcb.tile([P, 1], FP32, name=f"fcb{i}", tag="fcb")
            nc.sync.dma_start(c_t[:], fcb_d[i * P : (i + 1) * P, :])
            fcb_cols.append(c_t)
        identity = p_ones.tile([P, P], FP32, name="ident", tag="ident")
        nc.gpsimd.memset(identity[:], 0.0)
        nc.gpsimd.affine_select(
            out=identity[:], in_=identity[:], compare_op=ALU.not_equal,
            fill=1.0, base=0, pattern=[[-1, P]], channel_multiplier=1,
        )
        ones_row = p_ones.tile([1, P], FP32R, name="ones_row", tag="ones_row")
        nc.scalar.activation(ones_row[:], identity[0:1, :], AF.Identity, bias=1.0, scale=0.0)

        def load_x1t_chunks(tg, who):
            chunks = []
            for ec in range(DC):
                c = p_x1tc.tile([P, SB], FP32R, name=f"x1tc_{who}_{tg}_{ec}", tag="x1tc")
                nc.sync.dma_start(
                    c[:], x1t_d[ec * P : (ec + 1) * P, tg * SB : (tg + 1) * SB]
                )
                chunks.append(c)
            return chunks

        # ---------- kb prepass: kb = x1 @ bias as per-t' columns ----------
        # kb_row[tg] = bias_colchunks.T @ x1t chunks; then K=1 transposes to cols.
        kb_rows = []
        for tg in range(S // SB):
            chunks = load_x1t_chunks(tg, "kb")
            ps_kb = p_psum.tile([1, SB], FP32, name=f"pskb{tg}", tag="ps")
            for ec in range(DC):
                nc.tensor.matmul(
                    ps_kb[:], bias_cols[ec][:], chunks[ec][:],
                    start=(ec == 0), stop=(ec == DC - 1),
                )
            kb_r = p_row.tile([1, SB], FP32R, name=f"kbrow{tg}", tag=f"kbrow{tg}")
            nc.vector.tensor_copy(kb_r[:], ps_kb[:])
            kb_rows.append(kb_r)
        kb_cols = []
        for ti in range(TC):
            tg, sub = ti // 4, ti % 4
            ps_c = p_psum.tile([P, 4], FP32, name=f"pskc{ti}", tag="ps")
            nc.tensor.matmul(
                ps_c[:], kb_rows[tg][0:1, sub * P : (sub + 1) * P],
                ones_row[0:1, 0:4], start=True, stop=True,
            )
            kb_c = p_kb.tile([P, 1], FP32, name=f"kb{ti}", tag="kb")
            nc.vector.tensor_copy(kb_c[:], ps_c[:, 0:1])
            kb_cols.append(kb_c)

        # ---------- MM1 emitter ----------
        def emit_mm1(sb):
            s0 = sb * SB
            x2t_tiles = []
            for dc in range(DC):
                x2_t = p_x2t.tile([P, SB], FP32R, name=f"x2t_{sb}_{dc}", tag="x2t")
                nc.sync.dma_start(x2_t[:], x2t_d[dc * P : (dc + 1) * P, s0 : s0 + SB])
                x2t_tiles.append(x2_t)
            tt_tiles = []
            for eh in range(2):
                ps_t = [
                    p_psum.tile([P, SB], FP32, name=f"pst{sb}_{eh}_{i}", tag="ps")
                    for i in range(4)
                ]
                for dc in range(DC):
                    for i in range(4):
                        et = eh * 4 + i
                        nc.tensor.matmul(
                            ps_t[i][:],
                            u_tiles[dc][:, et * P : (et + 1) * P],
                            x2t_tiles[dc][:],
                            start=(dc == 0), stop=(dc == DC - 1),
                        )
                for i in range(4):
                    t_t = p_tt.tile([P, SB], FP32R, name=f"tt{sb}_{eh}_{i}", tag="tt")
                    nc.vector.tensor_copy(t_t[:], ps_t[i][:])
                    tt_tiles.append(t_t)
            return tt_tiles

        next_tt = emit_mm1(0)

        # ---------- resident x1 (bf16) for MM4 stationaries ----------
        x1_tiles = []
        for i in range(TC):
            x1_t = p_x1.tile([P, D], BF16, name=f"x1r{i}", tag="x1r")
            nc.sync.dma_start(x1_t[:], x1_d[i * P : (i + 1) * P, :])
            x1_tiles.append(x1_t)

        for sb in range(NSB):
            s0 = sb * SB
            tt_tiles = next_tt

            # ---- MM2: scoresT tiles + running elementwise max ----
            sc_tiles = []
            maxacc = p_aux.tile([P, SB], FP32, name=f"maxacc{sb}", tag="maxacc")
            chunks = None
            for ti in range(TC):
                tg, sub = ti // 4, ti % 4
                if sub == 0:
                    chunks = load_x1t_chunks(tg, f"s{sb}")
                ps_s = p_psum.tile([P, SB], FP32, name=f"pss{sb}_{ti}", tag="ps")
                for ec in range(DC):
                    nc.tensor.matmul(
                        ps_s[:],
                        chunks[ec][:, sub * P : (sub + 1) * P],
                        tt_tiles[ec][:],
                        start=(ec == 0), stop=(ec == DC - 1),
                    )
                s_t = p_sc.tile([P, SB], FP32, name=f"sc{sb}_{ti}", tag="sc")
                nc.vector.tensor_copy(s_t[:], ps_s[:])
                if ti == 0:
                    nc.scalar.copy(maxacc[:], ps_s[:])
                else:
                    nc.vector.tensor_max(maxacc[:], maxacc[:], ps_s[:])
                sc_tiles.append(s_t)

            # pipeline: next superblock's MM1 runs during the softmax phase
            if sb + 1 < NSB:
                next_tt = emit_mm1(sb + 1)

            # ---- per-s max over partitions: transpose + free reduce ----
            mrow = p_row.tile([1, SB], FP32R, name=f"mrow{sb}", tag="mrow")
            for blk in range(SB // P):
                ps_tr = p_psum.tile([P, P], FP32, name=f"ptr{sb}_{blk}", tag="ps")
                nc.tensor.transpose(
                    ps_tr[:], maxacc[:, blk * P : (blk + 1) * P], identity[:]
                )
                mcol = p_row.tile([P, 1], FP32, name=f"mcol{sb}_{blk}", tag="mcol")
                nc.vector.reduce_max(mcol[:], ps_tr[:], axis=AX.X)
                ps_rr = p_psum.tile([1, P], FP32, name=f"prr{sb}_{blk}", tag="ps")
                nc.tensor.transpose(ps_rr[:], mcol[:], identity[:])
                nc.vector.tensor_copy(mrow[:, blk * P : (blk + 1) * P], ps_rr[:])
            ps_mb = p_psum.tile([P, SB], FP32, name=f"pmb{sb}", tag="ps")
            nc.tensor.matmul(ps_mb[:], ones_row[:], mrow[:], start=True, stop=True)
            maxb = p_aux.tile([P, SB], FP32, name=f"maxb{sb}", tag="maxb")
            nc.vector.tensor_copy(maxb[:], ps_mb[:])

            # ---- exp(scores - maxb + kb) -> bf16 p tiles; running sum ----
            pb_tiles = []
            sumacc = p_aux.tile([P, SB], FP32, name=f"sumacc{sb}", tag="sumacc")
            for ti in range(TC):
                nc.vector.tensor_sub(sc_tiles[ti][:], sc_tiles[ti][:], maxb[:])
                p_t = p_pb.tile([P, SB], BF16, name=f"pb{sb}_{ti}", tag="pb")
                nc.scalar.activation(
                    p_t[:], sc_tiles[ti][:], AF.Exp, bias=kb_cols[ti][:], scale=1.0
                )
                if ti == 0:
                    nc.scalar.copy(sumacc[:], p_t[:])
                else:
                    nc.vector.tensor_add(sumacc[:], sumacc[:], p_t[:])
                pb_tiles.append(p_t)

            # ---- per-s sum over partitions: transpose + free reduce ----
            srow = p_row.tile([1, SB], FP32, name=f"srow{sb}", tag="srow")
            for blk in range(SB // P):
                ps_tr2 = p_psum.tile([P, P], FP32, name=f"ptr2{sb}_{blk}", tag="ps")
                nc.tensor.transpose(
                    ps_tr2[:], sumacc[:, blk * P : (blk + 1) * P], identity[:]
                )
                scol = p_row.tile([P, 1], FP32, name=f"scol{sb}_{blk}", tag="scol")
                nc.vector.reduce_sum(scol[:], ps_tr2[:], axis=AX.X)
                ps_rr2 = p_psum.tile([1, P], FP32, name=f"prr2{sb}_{blk}", tag="ps")
                nc.tensor.transpose(ps_rr2[:], scol[:], identity[:])
                nc.vector.tensor_copy(srow[:, blk * P : (blk + 1) * P], ps_rr2[:])
            rrow = p_row.tile([1, SB], FP32R, name=f"rrow{sb}", tag="rrow")
            with nc.allow_low_precision(reason="recip feeds fp32r matmul; fp22 ok"):
                nc.vector.reciprocal(rrow[:], srow[:])

            # ---- MM4 (bf16): oT = (p~ @ x1)^T ----
            ot_tiles = []
            for et in range(DC):
                ps_o = p_psum.tile([P, SB], FP32, name=f"pso{sb}_{et}", tag="ps")
                for ti in range(TC):
                    nc.tensor.matmul(
                        ps_o[:],
                        x1_tiles[ti][:, et * P : (et + 1) * P],
                        pb_tiles[ti][:],
                        start=(ti == 0), stop=(ti == TC - 1),
                    )
                o_t = p_ot.tile([P, SB], FP32R, name=f"ot{sb}_{et}", tag="ot")
                nc.vector.tensor_copy(o_t[:], ps_o[:])
                ot_tiles.append(o_t)

            # recip broadcast (emitted after MM4 so the PE isn't head-of-line
            # blocked on the DVE reciprocal)
            ps_rb = p_psum.tile([P, SB], FP32, name=f"prb{sb}", tag="ps")
            nc.tensor.matmul(ps_rb[:], ones_row[:], rrow[:], start=True, stop=True)
            recipb = p_aux.tile([P, SB], FP32, name=f"recipb{sb}", tag="recipb")
            nc.vector.tensor_copy(recipb[:], ps_rb[:])

            # ---- MM5 + normalize + bias + relu + store ----
            for ft in range(F // P):
                ps_f = p_psum.tile([P, SB], FP32, name=f"psf{sb}_{ft}", tag="ps")
                for ec in range(DC):
                    fcw_t = p_fcw.tile([P, P], FP32R, name=f"fcw{sb}_{ft}_{ec}", tag="fcw")
                    nc.sync.dma_start(
                        fcw_t[:], fcwt_d[ec * P : (ec + 1) * P, ft * P : (ft + 1) * P]
                    )
                    nc.tensor.matmul(
                        ps_f[:], fcw_t[:], ot_tiles[ec][:],
                        start=(ec == 0), stop=(ec == DC - 1),
                    )
                tmp = p_out.tile([P, SB], FP32, name=f"tmp{sb}_{ft}", tag="tmp")
                nc.vector.tensor_mul(tmp[:], ps_f[:], recipb[:])
                o_out = p_out.tile([P, SB], FP32, name=f"oo{sb}_{ft}", tag="oo")
                nc.scalar.activation(
                    o_out[:], tmp[:], AF.Relu, bias=fcb_cols[ft][:], scale=1.0
                )
                nc.sync.dma_start(outt_d[ft * P : (ft + 1) * P, s0 : s0 + SB], o_out[:])

    nc.compile()
    return nc


_NC_CACHE = None


def _get_nc():
    global _NC_CACHE
    if _NC_CACHE is None:
        _NC_CACHE = build_nc()
    return _NC_CACHE


def make_in_maps(x1, x2, U, bias, fc_w, fc_b):
    import ml_dtypes

    x1 = np.ascontiguousarray(np.asarray(x1, dtype=np.float32))
    x2 = np.ascontiguousarray(np.asarray(x2, dtype=np.float32))
    U = np.ascontiguousarray(np.asarray(U, dtype=np.float32))
    bias = np.asarray(bias, dtype=np.float32).reshape(D, 1)
    fc_w = np.asarray(fc_w, dtype=np.float32)
    fc_b = np.asarray(fc_b, dtype=np.float32).reshape(F, 1)
    fcwt = np.ascontiguousarray(fc_w.T)
    in_maps = []
    for b in range(B):
        in_maps.append(
            {
                "x1": np.ascontiguousarray(x1[b].astype(ml_dtypes.bfloat16)),
                "x1t": np.ascontiguousarray(x1[b].T),
                "x2t": np.ascontiguousarray(x2[b].T),
                "u": U,
                "fcwt": fcwt,
                "bias": bias,
                "fcb": fc_b,
            }
        )
    return in_maps


def kernel(x1, x2, U, bias, fc_w, fc_b):
    from concourse.bass_utils import run_bass_kernel_spmd

    nc = _get_nc()
    in_maps = make_in_maps(x1, x2, U, bias, fc_w, fc_b)
    res = run_bass_kernel_spmd(nc, in_maps, core_ids=list(range(B)))
    out = np.stack([np.ascontiguousarray(r["outt"].T) for r in res.results])
    return out.astype(np.float32)



# revision 11
# speedup vs baseline: 1.5263x; 1.5263x over previous
"""BiaffineAttn Trainium2 kernel.

Math (per batch b):
    t    = x2 @ U + bias[None, :]      [S, D]   (bias folded: x2 U x1^T + 1 (x1 bias)^T
                                                 == [x2|1] [[U];[bias^T]] x1^T)
    attn = t @ x1^T
    p    = softmax(attn, axis=-1)
    out  = relu((p @ x1) @ fc_w^T + fc_b)       [S, F]

Sharding: data-parallel over batch B=8, one batch per NeuronCore.

Per-core pipeline, transposed orientation (softmax key dim t' on partitions),
software-pipelined across S-superblocks so the PE never idles during softmax:

  prologue:  MM1(0) -> tt0,  MM2(0) -> scores0 + running max,  MM1(1)
  block(sb): max partition-reduce (PE transposes + DVE) -> maxb broadcast
             interleave{ exp(sb) on DVE/Scalar | MM2(sb+1) on PE }   <- gap filler
             MM1(sb+2), rowsum via ones-matmul, fast reciprocal
             MM4(sb) (p @ x1)^T, recip broadcast, MM5 + relu + store
  block(3):  exp(3) interleaved with MM4(3) in ti-outer order (8 held PSUM banks)

All big matmuls stream at 1 cycle/row (fp32r or bf16 moving, free dim 512).
x1t chunks for MM2 stationaries are streamed from DRAM with ~1.5-tg lookahead,
issued on the (otherwise idle) GpSimd queue; fc_w and x1(bf16) are resident.

Host side: transposes x1/x2/fc_w per-core and transposes the [F,S] per-core
output back to [S,F] when gathering (fp32 DMA transpose does not exist on TRN2).
"""

import os
import sys
from contextlib import ExitStack

import numpy as np

for _p in ("/opt/trn_rl_repo", os.path.expanduser("~/.axon_site/_ro/trn_rl_repo")):
    if os.path.isdir(_p) and _p not in sys.path:
        sys.path.insert(0, _p)

import concourse.bass as bass
import concourse.mybir as mybir
import concourse.tile as tile
from concourse import bacc

B = 8
S = 2048          # sequence length (both s and t')
D = 1024          # d_model
F = 512           # fc output dim
P = 128
SB = 512          # s superblock (moving free dim of every matmul)
NSB = S // SB     # 4
DC = D // P       # 8 contraction chunks of d / e
TC = S // P       # 16 t' tiles
NTG = NSB         # 4 t' groups of 4 tiles
FT = F // P       # 4
FP32 = mybir.dt.float32
FP32R = mybir.dt.float32r
BF16 = mybir.dt.bfloat16
AF = mybir.ActivationFunctionType
ALU = mybir.AluOpType
AX = mybir.AxisListType

OT_DT = BF16      # dtype of MM4 output tiles (MM5 moving operand)


def build_nc():
    nc = bacc.Bacc(
        "TRN2",
        target_bir_lowering=False,
        debug=False,
        enable_asserts=False,
    )

    x1_d = nc.dram_tensor("x1", [S, D], BF16, kind="ExternalInput")
    x1t_d = nc.dram_tensor("x1t", [D, S], FP32R, kind="ExternalInput")
    x2t_d = nc.dram_tensor("x2t", [D, S], FP32R, kind="ExternalInput")
    u_d = nc.dram_tensor("u", [D, D], FP32R, kind="ExternalInput")
    fcwt_d = nc.dram_tensor("fcwt", [D, F], BF16, kind="ExternalInput")
    bias_d = nc.dram_tensor("bias", [D, 1], FP32, kind="ExternalInput")
    fcb_d = nc.dram_tensor("fcb", [F, 1], FP32, kind="ExternalInput")
    outt_d = nc.dram_tensor("outt", [F, S], FP32, kind="ExternalOutput")

    with tile.TileContext(nc) as tc, ExitStack() as ctx:
        # ---------- pools ----------
        p_u = ctx.enter_context(tc.tile_pool(name="ures", bufs=DC))
        p_x1 = ctx.enter_context(tc.tile_pool(name="x1res", bufs=TC))
        p_bc = ctx.enter_context(tc.tile_pool(name="biascols", bufs=DC))
        p_fcb = ctx.enter_context(tc.tile_pool(name="fcbcols", bufs=FT))
        p_fcw = ctx.enter_context(tc.tile_pool(name="fcwres", bufs=DC))
        p_ones = ctx.enter_context(tc.tile_pool(name="ones", bufs=1))
        p_psum = ctx.enter_context(tc.tile_pool(name="psum", bufs=8, space="PSUM"))
        p_x2t = ctx.enter_context(tc.tile_pool(name="x2ts", bufs=6))
        p_x1tc = ctx.enter_context(tc.tile_pool(name="x1tcs", bufs=11))
        p_tt = ctx.enter_context(tc.tile_pool(name="tts", bufs=10))
        p_sc = ctx.enter_context(tc.tile_pool(name="scores", bufs=TC))
        p_pb = ctx.enter_context(tc.tile_pool(name="pbf", bufs=TC))
        p_ot = ctx.enter_context(tc.tile_pool(name="ots", bufs=DC))
        p_aux = ctx.enter_context(tc.tile_pool(name="aux", bufs=2))
        p_aux1 = ctx.enter_context(tc.tile_pool(name="aux1", bufs=1))
        p_row = ctx.enter_context(tc.tile_pool(name="rows", bufs=1))
        p_tmp = ctx.enter_context(tc.tile_pool(name="tmps", bufs=1))
        p_oo = ctx.enter_context(tc.tile_pool(name="oos", bufs=2))

        # ---------- prologue DMAs: MM1(0) critical path first ----------
        u_tiles = []
        x2t_tiles = {0: []}
        for dc in range(DC):
            u_t = p_u.tile([P, D], FP32R, name=f"ur{dc}", tag="ur")
            nc.sync.dma_start(u_t[:], u_d[dc * P : (dc + 1) * P, :])
            u_tiles.append(u_t)
            x2_t = p_x2t.tile([P, SB], FP32R, name=f"x2t0_{dc}", tag="x2t")
            nc.sync.dma_start(x2_t[:], x2t_d[dc * P : (dc + 1) * P, 0:SB])
            x2t_tiles[0].append(x2_t)
        bias_cols = []
        for dc in range(DC):
            b_t = p_bc.tile([P, 1], FP32, name=f"bc{dc}", tag="bc")
            nc.sync.dma_start(b_t[:], bias_d[dc * P : (dc + 1) * P, :])
            bias_cols.append(b_t)

        identity32 = p_ones.tile([P, P], FP32, name="ident32", tag="ident32")
        nc.gpsimd.memset(identity32[:], 0.0)
        nc.gpsimd.affine_select(
            out=identity32[:], in_=identity32[:], compare_op=ALU.not_equal,
            fill=1.0, base=0, pattern=[[-1, P]], channel_multiplier=1,
        )
        identity = p_ones.tile([P, P], FP32R, name="ident", tag="ident")
        nc.scalar.activation(identity[:], identity32[:], AF.Identity, bias=0.0, scale=1.0)
        ones_row = p_ones.tile([1, P], FP32R, name="ones_row", tag="ones_row")
        nc.scalar.activation(ones_row[:], identity[0:1, :], AF.Identity, bias=1.0, scale=0.0)
        ones_col = p_ones.tile([P, 1], FP32R, name="ones_col", tag="ones_col")
        nc.scalar.activation(ones_col[:], identity[:, 0:1], AF.Identity, bias=1.0, scale=0.0)
        ones_row32 = p_ones.tile([1, P], FP32, name="ones_row32", tag="ones_row32")
        nc.scalar.activation(ones_row32[:], identity[0:1, :], AF.Identity, bias=1.0, scale=0.0)

        # x1t chunk streaming (MM2 stationaries), issued on the idle GpSimd queue
        def issue_x1tc(sb, tg):
            chunks = []
            for ec in range(DC):
                c = p_x1tc.tile([P, SB], FP32R, name=f"x1tc{sb}_{tg}_{ec}", tag="x1tc")
                nc.gpsimd.dma_start(
                    c[:], x1t_d[ec * P : (ec + 1) * P, tg * SB : (tg + 1) * SB]
                )
                chunks.append(c)
            return chunks

        x1tc = {0: {}}
        x1tc[0][0] = issue_x1tc(0, 0)
        x1tc[0][1] = issue_x1tc(0, 1)

        def issue_x2t(sb):
            tiles = []
            for dc in range(DC):
                x2_t = p_x2t.tile([P, SB], FP32R, name=f"x2t{sb}_{dc}", tag="x2t")
                nc.sync.dma_start(
                    x2_t[:], x2t_d[dc * P : (dc + 1) * P, sb * SB : (sb + 1) * SB]
                )
                tiles.append(x2_t)
            return tiles

        # ---------- MM1: ttT = (x2 @ U)^T + bias, 8 held PSUM banks ----------
        def emit_mm1(sb):
            ps = [
                p_psum.tile([P, SB], FP32, name=f"ps1_{sb}_{et}", tag="ps")
                for et in range(DC)
            ]
            for dc in range(DC):
                for et in range(DC):
                    nc.tensor.matmul(
                        ps[et][:],
                        u_tiles[dc][:, et * P : (et + 1) * P],
                        x2t_tiles[sb][dc][:],
                        start=(dc == 0), stop=(dc == DC - 1),
                    )
            tts = []
            for et in range(DC):
                t_t = p_tt.tile([P, SB], FP32R, name=f"tt{sb}_{et}", tag="tt")
                nc.scalar.activation(
                    t_t[:], ps[et][:], AF.Identity, bias=bias_cols[et][:], scale=1.0
                )
                tts.append(t_t)
            return tts

        tt = {0: emit_mm1(0)}

        # resident x1 (bf16, MM4 stationaries) + fcw + fcb — off critical path
        x1_tiles = []
        for i in range(TC):
            x1_t = p_x1.tile([P, D], BF16, name=f"x1r{i}", tag="x1r")
            nc.sync.dma_start(x1_t[:], x1_d[i * P : (i + 1) * P, :])
            x1_tiles.append(x1_t)
        fcw_tiles = []
        for ec in range(DC):
            f_t = p_fcw.tile([P, F], BF16, name=f"fcw{ec}", tag="fcw")
            nc.sync.dma_start(f_t[:], fcwt_d[ec * P : (ec + 1) * P, :])
            fcw_tiles.append(f_t)
        fcb_cols = []
        for i in range(FT):
            c_t = p_fcb.tile([P, 1], FP32, name=f"fcb{i}", tag="fcb")
            nc.sync.dma_start(c_t[:], fcb_d[i * P : (i + 1) * P, :])
            fcb_cols.append(c_t)

        # ---------- MM2 per-tile emitter: scoresT tile + running max ----------
        sc_tiles = {}
        maxacc = {}
        sumacc = {}
        pb_tiles = {}

        def emit_mm2_tile(sb, ti):
            tg, sub = divmod(ti, NTG)
            if ti == 0:
                sc_tiles[sb] = []
                maxacc[sb] = p_aux.tile(
                    [P, SB], FP32R, name=f"maxacc{sb}", tag="maxacc"
                )
            ps_s = p_psum.tile([P, SB], FP32, name=f"pss{sb}_{ti}", tag="ps")
            chunks = x1tc[sb][tg]
            for ec in range(DC):
                nc.tensor.matmul(
                    ps_s[:],
                    chunks[ec][:, sub * P : (sub + 1) * P],
                    tt[sb][ec][:],
                    start=(ec == 0), stop=(ec == DC - 1),
                )
            s_t = p_sc.tile([P, SB], FP32, name=f"sc{sb}_{ti}", tag="sc")
            nc.vector.tensor_copy(s_t[:], ps_s[:])
            if ti == 0:
                nc.vector.tensor_copy(maxacc[sb][:], ps_s[:])
            else:
                nc.vector.tensor_max(maxacc[sb][:], maxacc[sb][:], ps_s[:])
            sc_tiles[sb].append(s_t)

        # prologue MM2(0) with chunk prefetch, then MM1(1)
        for ti in range(TC):
            tg, sub = divmod(ti, NTG)
            if sub == 0 and tg + 2 < NTG:
                x1tc[0][tg + 2] = issue_x1tc(0, tg + 2)
            emit_mm2_tile(0, ti)
        x2t_tiles[1] = issue_x2t(1)
        tt[1] = emit_mm1(1)

        # ---------- softmax helpers ----------
        def emit_max_reduce(sb):
            """per-s max over partitions -> maxb [P,SB] broadcast tile."""
            mrow = p_row.tile([1, SB], FP32R, name=f"mrow{sb}", tag="mrow")
            trs, mcols = [], []
            for blk in range(SB // P):
                ps_tr = p_psum.tile([P, P], FP32R, name=f"ptr{sb}_{blk}", tag="ps")
                nc.tensor.transpose(
                    ps_tr[:], maxacc[sb][:, blk * P : (blk + 1) * P], identity[:]
                )
                trs.append(ps_tr)
            for blk in range(SB // P):
                mcol = p_row.tile([P, 1], FP32R, name=f"mcol{sb}_{blk}", tag=f"mcol{blk % 2}")
                nc.vector.reduce_max(mcol[:], trs[blk][:], axis=AX.X)
                mcols.append(mcol)
            ps_rrs = []
            for blk in range(SB // P):
                ps_rr = p_psum.tile([1, P], FP32R, name=f"prr{sb}_{blk}", tag="ps")
                nc.tensor.transpose(ps_rr[:], mcols[blk][:], identity[:])
                ps_rrs.append(ps_rr)
            for blk in range(SB // P):
                nc.vector.tensor_copy(mrow[:, blk * P : (blk + 1) * P], ps_rrs[blk][:])
            ps_mb = p_psum.tile([P, SB], FP32, name=f"pmb{sb}", tag="ps")
            nc.tensor.matmul(ps_mb[:], ones_row[:], mrow[:], start=True, stop=True)
            maxb = p_aux1.tile([P, SB], FP32, name=f"maxb{sb}", tag="maxb")
            nc.vector.tensor_copy(maxb[:], ps_mb[:])
            return maxb

        def emit_exp_tile(sb, ti, maxb):
            if ti == 0:
                pb_tiles[sb] = []
                sumacc[sb] = p_aux.tile(
                    [P, SB], FP32R, name=f"sumacc{sb}", tag="sumacc"
                )
            s_t = sc_tiles[sb][ti]
            nc.vector.tensor_sub(s_t[:], s_t[:], maxb[:])
            p_t = p_pb.tile([P, SB], BF16, name=f"pb{sb}_{ti}", tag="pb")
            nc.scalar.activation(p_t[:], s_t[:], AF.Exp, bias=0.0, scale=1.0)
            if ti == 0:
                nc.vector.tensor_copy(sumacc[sb][:], p_t[:])
            else:
                nc.vector.tensor_add(sumacc[sb][:], sumacc[sb][:], p_t[:])
            pb_tiles[sb].append(p_t)

        def emit_sum_recip(sb):
            """rowsum over partitions via ones-matmul, then fast reciprocal."""
            ps_sum = p_psum.tile([1, SB], FP32, name=f"psum{sb}", tag="ps")
            nc.tensor.matmul(ps_sum[:], ones_col[:], sumacc[sb][:], start=True, stop=True)
            rrow = p_row.tile([1, SB], FP32, name=f"rrow{sb}", tag="rrow")
            with nc.allow_low_precision(reason="softmax denom reciprocal; fp22 ok"):
                nc.vector.reciprocal_approx_fast(rrow[:], ps_sum[:])
            return rrow

        def emit_recip_bcast(sb, rrow):
            ps_rb = p_psum.tile([P, SB], FP32, name=f"prb{sb}", tag="ps")
            nc.tensor.matmul(ps_rb[:], ones_row32[:], rrow[:], start=True, stop=True)
            recipb = p_aux1.tile([P, SB], FP32, name=f"recipb{sb}", tag="recipb")
            nc.vector.tensor_copy(recipb[:], ps_rb[:])
            return recipb

        def emit_mm4_et_outer(sb):
            ots = []
            for et in range(DC):
                ps_o = p_psum.tile([P, SB], FP32, name=f"pso{sb}_{et}", tag="ps")
                for ti in range(TC):
                    nc.tensor.matmul(
                        ps_o[:],
                        x1_tiles[ti][:, et * P : (et + 1) * P],
                        pb_tiles[sb][ti][:],
                        start=(ti == 0), stop=(ti == TC - 1),
                    )
                o_t = p_ot.tile([P, SB], OT_DT, name=f"ot{sb}_{et}", tag="ot")
                nc.vector.tensor_copy(o_t[:], ps_o[:])
                ots.append(o_t)
            return ots

        def emit_mm5(sb, ots, recipb):
            s0 = sb * SB
            for ft in range(FT):
                ps_f = p_psum.tile([P, SB], FP32, name=f"psf{sb}_{ft}", tag="ps")
                for ec in range(DC):
                    nc.tensor.matmul(
                        ps_f[:],
                        fcw_tiles[ec][:, ft * P : (ft + 1) * P],
                        ots[ec][:],
                        start=(ec == 0), stop=(ec == DC - 1),
                    )
                tmp = p_tmp.tile([P, SB], FP32, name=f"tmp{sb}_{ft}", tag="tmp")
                nc.vector.tensor_mul(tmp[:], ps_f[:], recipb[:])
                o_out = p_oo.tile([P, SB], FP32, name=f"oo{sb}_{ft}", tag="oo")
                nc.scalar.activation(
                    o_out[:], tmp[:], AF.Relu, bias=fcb_cols[ft][:], scale=1.0
                )
                nc.sync.dma_start(outt_d[ft * P : (ft + 1) * P, s0 : s0 + SB], o_out[:])

        # ---------- steady-state blocks ----------
        for sb in range(NSB):
            if sb + 2 < NSB:
                x2t_tiles[sb + 2] = issue_x2t(sb + 2)
            if sb + 1 < NSB:
                x1tc[sb + 1] = {0: issue_x1tc(sb + 1, 0)}

            maxb = emit_max_reduce(sb)

            if sb + 1 < NSB:
                # PE chews MM2(sb+1) while DVE/Scalar run exp(sb)
                for ti in range(TC):
                    emit_exp_tile(sb, ti, maxb)
                    tg, sub = divmod(ti, NTG)
                    if sub == 0 and tg + 1 < NTG:
                        x1tc[sb + 1][tg + 1] = issue_x1tc(sb + 1, tg + 1)
                    emit_mm2_tile(sb + 1, ti)
                if sb + 2 < NSB:
                    tt[sb + 2] = emit_mm1(sb + 2)
                rrow = emit_sum_recip(sb)
                ots = emit_mm4_et_outer(sb)
                recipb = emit_recip_bcast(sb, rrow)
                emit_mm5(sb, ots, recipb)
            else:
                # last block: PE chases exp down the tiles (ti-outer MM4)
                mm4_ps = [
                    p_psum.tile([P, SB], FP32, name=f"pso{sb}_{et}", tag="ps")
                    for et in range(DC)
                ]
                for ti in range(TC):
                    emit_exp_tile(sb, ti, maxb)
                    for et in range(DC):
                        nc.tensor.matmul(
                            mm4_ps[et][:],
                            x1_tiles[ti][:, et * P : (et + 1) * P],
                            pb_tiles[sb][ti][:],
                            start=(ti == 0), stop=(ti == TC - 1),
                        )
                ots = []
                for et in range(DC):
                    o_t = p_ot.tile([P, SB], OT_DT, name=f"ot{sb}_{et}", tag="ot")
                    nc.vector.tensor_copy(o_t[:], mm4_ps[et][:])
                    ots.append(o_t)
                rrow = emit_sum_recip(sb)
                recipb = emit_recip_bcast(sb, rrow)
                emit_mm5(sb, ots, recipb)

    nc.compile()
    return nc


_NC_CACHE = None


def _get_nc():
    global _NC_CACHE
    if _NC_CACHE is None:
        _NC_CACHE = build_nc()
    return _NC_CACHE


def make_in_maps(x1, x2, U, bias, fc_w, fc_b):
    import ml_dtypes

    x1 = np.ascontiguousarray(np.asarray(x1, dtype=np.float32))
    x2 = np.ascontiguousarray(np.asarray(x2, dtype=np.float32))
    U = np.ascontiguousarray(np.asarray(U, dtype=np.float32))
    bias = np.asarray(bias, dtype=np.float32).reshape(D, 1)
    fc_w = np.asarray(fc_w, dtype=np.float32)
    fc_b = np.asarray(fc_b, dtype=np.float32).reshape(F, 1)
    fcwt = np.ascontiguousarray(fc_w.T.astype(ml_dtypes.bfloat16))
    in_maps = []
    for b in range(B):
        in_maps.append(
            {
                "x1": np.ascontiguousarray(x1[b].astype(ml_dtypes.bfloat16)),
                "x1t": np.ascontiguousarray(x1[b].T),
                "x2t": np.ascontiguousarray(x2[b].T),
                "u": U,
                "fcwt": fcwt,
                "bias": bias,
                "fcb": fc_b,
            }
        )
    return in_maps


def kernel(x1, x2, U, bias, fc_w, fc_b):
    from concourse.bass_utils import run_bass_kernel_spmd

    nc = _get_nc()
    in_maps = make_in_maps(x1, x2, U, bias, fc_w, fc_b)
    res = run_bass_kernel_spmd(nc, in_maps, core_ids=list(range(B)))
    out = np.stack([np.ascontiguousarray(r["outt"].T) for r in res.results])
    return out.astype(np.float32)


# revision 17
# speedup vs baseline: 1.5645x; 1.0251x over previous
"""BiaffineAttn Trainium2 kernel.

Math (per batch b):
    t    = x2 @ U + bias[None, :]      [S, D]   (bias folded: x2 U x1^T + 1 (x1 bias)^T
                                                 == [x2|1] [[U];[bias^T]] x1^T)
    attn = t @ x1^T
    p    = softmax(attn, axis=-1)
    out  = relu((p @ x1) @ fc_w^T + fc_b)       [S, F]

Sharding: data-parallel over batch B=8, one batch per NeuronCore.

Per-core pipeline, transposed orientation (softmax key dim t' on partitions),
software-pipelined across S-superblocks so the PE never idles during softmax.
The interleave window overlays three instruction streams per t'-tile:
  PE:     MM2(sb+1) tile (8 mm, 4-buf PSUM ring) + one MM1(sb+2) dc-step
          (4 mm into 4 held PSUM banks)  -> 12 mm/tile keeps PE the pacer
  Scalar: exp(sb, tile) + MM2 psum->SBUF score copy
  DVE:    running max + softmax-denominator accumulation
  GpSimd: score - maxb subtract (SBUF-only; Pool cannot touch PSUM)
Block 2 has no MM1 to interleave, so half of MM4(2) (et 0-3, ti-outer) fills
the window; block 3 interleaves the full ti-outer MM4(3) in 8 held banks.

rowsum via single ones-column matmul; reciprocal_approx_fast; relu+bias via
Scalar activation; [F,SB] stores per superblock.

All tensors are host-relaid so every resident loads in ONE big DMA (32KB
rows) and each x1t t'-group is a single [128, 4096] transfer (2-deep ring).

Host side: builds the relaid views and transposes the [F,S] per-core output
back to [S,F] when gathering (fp32 DMA transpose does not exist on TRN2).
"""

import os
import sys
from contextlib import ExitStack

import numpy as np

for _p in ("/opt/trn_rl_repo", os.path.expanduser("~/.axon_site/_ro/trn_rl_repo")):
    if os.path.isdir(_p) and _p not in sys.path:
        sys.path.insert(0, _p)

import concourse.bass as bass
import concourse.mybir as mybir
import concourse.tile as tile
from concourse import bacc

B = 8
S = 2048          # sequence length (both s and t')
D = 1024          # d_model
F = 512           # fc output dim
P = 128
SB = 512          # s superblock (moving free dim of every matmul)
NSB = S // SB     # 4
DC = D // P       # 8 contraction chunks of d / e
TC = S // P       # 16 t' tiles
NTG = NSB         # 4 t' groups of 4 tiles
FT = F // P       # 4
FP32 = mybir.dt.float32
FP32R = mybir.dt.float32r
BF16 = mybir.dt.bfloat16
AF = mybir.ActivationFunctionType
ALU = mybir.AluOpType
AX = mybir.AxisListType

OT_DT = BF16      # dtype of MM4 output tiles (MM5 moving operand)


def build_nc():
    nc = bacc.Bacc(
        "TRN2",
        target_bir_lowering=False,
        debug=False,
        enable_asserts=False,
    )

    # host-relaid tensors: row p holds the p-th partition's data for every tile
    x1_d = nc.dram_tensor("x1g", [P, TC * D], BF16, kind="ExternalInput")
    x1t_d = nc.dram_tensor("x1tg", [P, NTG * DC * SB], FP32R, kind="ExternalInput")
    x2t_d = nc.dram_tensor("x2t", [D, S], FP32R, kind="ExternalInput")
    u_d = nc.dram_tensor("ug", [P, DC * D], FP32R, kind="ExternalInput")
    fcwt_d = nc.dram_tensor("fcwg", [P, DC * F], BF16, kind="ExternalInput")
    bias_d = nc.dram_tensor("biasg", [P, DC], FP32, kind="ExternalInput")
    fcb_d = nc.dram_tensor("fcbg", [P, FT], FP32, kind="ExternalInput")
    outt_d = nc.dram_tensor("outt", [F, S], FP32, kind="ExternalOutput")

    with tile.TileContext(nc) as tc, ExitStack() as ctx:
        # ---------- pools ----------
        p_u = ctx.enter_context(tc.tile_pool(name="ures", bufs=1))
        p_x1 = ctx.enter_context(tc.tile_pool(name="x1res", bufs=1))
        p_bc = ctx.enter_context(tc.tile_pool(name="biascols", bufs=1))
        p_fcb = ctx.enter_context(tc.tile_pool(name="fcbcols", bufs=1))
        p_fcw = ctx.enter_context(tc.tile_pool(name="fcwres", bufs=1))
        p_ones = ctx.enter_context(tc.tile_pool(name="ones", bufs=1))
        p_psum = ctx.enter_context(tc.tile_pool(name="psum", bufs=4, space="PSUM"))
        p_psum1 = ctx.enter_context(tc.tile_pool(name="psum1", bufs=4, space="PSUM"))
        p_x2t = ctx.enter_context(tc.tile_pool(name="x2ts", bufs=5))
        p_x1tc = ctx.enter_context(tc.tile_pool(name="x1tgs", bufs=2))
        p_tt = ctx.enter_context(tc.tile_pool(name="tts", bufs=9))
        p_sc = ctx.enter_context(tc.tile_pool(name="scores", bufs=TC))
        p_pb = ctx.enter_context(tc.tile_pool(name="pbf", bufs=TC))
        p_ot = ctx.enter_context(tc.tile_pool(name="ots", bufs=DC))
        p_aux = ctx.enter_context(tc.tile_pool(name="aux", bufs=1))
        p_row = ctx.enter_context(tc.tile_pool(name="rows", bufs=1))
        p_tmp = ctx.enter_context(tc.tile_pool(name="tmps", bufs=1))
        p_oo = ctx.enter_context(tc.tile_pool(name="oos", bufs=1))

        # ---------- prologue DMAs: MM1(0) critical path first ----------
        u_big = p_u.tile([P, DC * D], FP32R, name="ug", tag="ur")
        for q in range(4):
            nc.sync.dma_start(
                u_big[:, q * 2 * D : (q + 1) * 2 * D],
                u_d[:, q * 2 * D : (q + 1) * 2 * D],
            )

        def u_sl(dc, et):
            return u_big[:, dc * D + et * P : dc * D + (et + 1) * P]

        x2t_tiles = {0: []}
        for dc in range(DC):
            x2_t = p_x2t.tile([P, SB], FP32R, name=f"x2t0_{dc}", tag="x2t")
            nc.sync.dma_start(x2_t[:], x2t_d[dc * P : (dc + 1) * P, 0:SB])
            x2t_tiles[0].append(x2_t)

        # x1t t'-group streaming (MM2 stationaries): one DMA per group, ring 2
        x1tg = {}

        def issue_x1tg(key, tg):
            t = p_x1tc.tile([P, DC * SB], FP32R, name=f"x1tg{key}", tag="x1tg")
            nc.sync.dma_start(t[:], x1t_d[:, tg * DC * SB : (tg + 1) * DC * SB])
            x1tg[key] = t

        issue_x1tg(0, 0)
        bias_cols = p_bc.tile([P, DC], FP32, name="bc", tag="bc")
        nc.sync.dma_start(bias_cols[:], bias_d[:, :])
        issue_x1tg(1, 1)

        x1_big = p_x1.tile([P, TC * D], BF16, name="x1g", tag="x1r")
        nc.sync.dma_start(x1_big[:], x1_d[:, :])

        def x1_sl(ti, et):
            return x1_big[:, ti * D + et * P : ti * D + (et + 1) * P]

        fcw_big = p_fcw.tile([P, DC * F], BF16, name="fcwg", tag="fcw")
        nc.sync.dma_start(fcw_big[:], fcwt_d[:, :])

        def fcw_sl(ec, ft):
            return fcw_big[:, ec * F + ft * P : ec * F + (ft + 1) * P]

        fcb_cols = p_fcb.tile([P, FT], FP32, name="fcb", tag="fcb")
        nc.sync.dma_start(fcb_cols[:], fcb_d[:, :])

        identity32 = p_ones.tile([P, P], FP32, name="ident32", tag="ident32")
        nc.gpsimd.memset(identity32[:], 0.0)
        nc.gpsimd.affine_select(
            out=identity32[:], in_=identity32[:], compare_op=ALU.not_equal,
            fill=1.0, base=0, pattern=[[-1, P]], channel_multiplier=1,
        )
        identity = p_ones.tile([P, P], FP32R, name="ident", tag="ident")
        nc.scalar.activation(identity[:], identity32[:], AF.Identity, bias=0.0, scale=1.0)
        ones_row = p_ones.tile([1, P], FP32R, name="ones_row", tag="ones_row")
        nc.scalar.activation(ones_row[:], identity[0:1, :], AF.Identity, bias=1.0, scale=0.0)
        ones_col = p_ones.tile([P, 1], FP32R, name="ones_col", tag="ones_col")
        nc.scalar.activation(ones_col[:], identity[:, 0:1], AF.Identity, bias=1.0, scale=0.0)
        ones_row32 = p_ones.tile([1, P], FP32, name="ones_row32", tag="ones_row32")
        nc.scalar.activation(ones_row32[:], identity[0:1, :], AF.Identity, bias=1.0, scale=0.0)

        def issue_x2t(sb):
            tiles = []
            for dc in range(DC):
                x2_t = p_x2t.tile([P, SB], FP32R, name=f"x2t{sb}_{dc}", tag="x2t")
                nc.sync.dma_start(
                    x2_t[:], x2t_d[dc * P : (dc + 1) * P, sb * SB : (sb + 1) * SB]
                )
                tiles.append(x2_t)
            return tiles

        x2t_tiles[1] = issue_x2t(1)

        # ---------- MM1: ttT = (x2 @ U)^T + bias, 8 held banks (both pools) ----------
        tt = {}

        def emit_mm1(sb):
            tt[sb] = []
            ps = [
                p_psum1.tile([P, SB], FP32, name=f"ps1_{sb}_{et}", tag="ps1")
                for et in range(4)
            ] + [
                p_psum.tile([P, SB], FP32, name=f"ps1b_{sb}_{et}", tag="ps")
                for et in range(4, DC)
            ]
            for dc in range(DC):
                for et in range(DC):
                    nc.tensor.matmul(
                        ps[et][:], u_sl(dc, et), x2t_tiles[sb][dc][:],
                        start=(dc == 0), stop=(dc == DC - 1),
                    )
            for et in range(DC):
                t_t = p_tt.tile([P, SB], FP32R, name=f"tt{sb}_{et}", tag="tt")
                nc.scalar.activation(
                    t_t[:], ps[et][:], AF.Identity,
                    bias=bias_cols[:, et : et + 1], scale=1.0,
                )
                tt[sb].append(t_t)

        emit_mm1(0)

        # ---------- MM2 per-tile emitter: scoresT tile + running max ----------
        sc_tiles = {}
        maxacc = {}
        sumacc = {}
        pb_tiles = {}

        def emit_mm2_tile(sb, ti):
            tg, sub = divmod(ti, SB // P)
            if ti == 0:
                sc_tiles[sb] = []
                maxacc[sb] = p_aux.tile(
                    [P, SB], FP32R, name=f"maxacc{sb}", tag="maxacc"
                )
            ps_s = p_psum.tile([P, SB], FP32, name=f"pss{sb}_{ti}", tag="ps")
            grp = x1tg[sb * NTG + tg]
            for ec in range(DC):
                nc.tensor.matmul(
                    ps_s[:],
                    grp[:, ec * SB + sub * P : ec * SB + (sub + 1) * P],
                    tt[sb][ec][:],
                    start=(ec == 0), stop=(ec == DC - 1),
                )
            s_t = p_sc.tile([P, SB], FP32, name=f"sc{sb}_{ti}", tag="sc")
            nc.scalar.copy(s_t[:], ps_s[:])
            if ti == 0:
                nc.vector.tensor_copy(maxacc[sb][:], s_t[:])
            else:
                nc.vector.tensor_max(maxacc[sb][:], maxacc[sb][:], s_t[:])
            sc_tiles[sb].append(s_t)

        # prologue MM2(0) with group prefetch, then MM1(1)
        for ti in range(TC):
            tg, sub = divmod(ti, SB // P)
            if sub == 0 and tg + 2 < NTG:
                issue_x1tg(tg + 2, tg + 2)
            emit_mm2_tile(0, ti)
        emit_mm1(1)

        # ---------- softmax helpers ----------
        def emit_max_reduce(sb):
            """per-s max over partitions -> maxb [P,SB] broadcast tile."""
            mrow = p_row.tile([1, SB], FP32R, name=f"mrow{sb}", tag="mrow")
            trs, mcols = [], []
            for blk in range(SB // P):
                ps_tr = p_psum.tile([P, P], FP32R, name=f"ptr{sb}_{blk}", tag="ps")
                nc.tensor.transpose(
                    ps_tr[:], maxacc[sb][:, blk * P : (blk + 1) * P], identity[:]
                )
                trs.append(ps_tr)
            for blk in range(SB // P):
                mcol = p_row.tile([P, 1], FP32R, name=f"mcol{sb}_{blk}", tag=f"mcol{blk % 2}")
                nc.vector.reduce_max(mcol[:], trs[blk][:], axis=AX.X)
                mcols.append(mcol)
            ps_rrs = []
            for blk in range(SB // P):
                ps_rr = p_psum.tile([1, P], FP32R, name=f"prr{sb}_{blk}", tag="ps")
                nc.tensor.transpose(ps_rr[:], mcols[blk][:], identity[:])
                ps_rrs.append(ps_rr)
            for blk in range(SB // P):
                nc.vector.tensor_copy(mrow[:, blk * P : (blk + 1) * P], ps_rrs[blk][:])
            ps_mb = p_psum.tile([P, SB], FP32, name=f"pmb{sb}", tag="ps")
            nc.tensor.matmul(ps_mb[:], ones_row[:], mrow[:], start=True, stop=True)
            maxb = p_aux.tile([P, SB], FP32, name=f"maxb{sb}", tag="maxb")
            nc.vector.tensor_copy(maxb[:], ps_mb[:])
            return maxb

        def emit_exp_tile(sb, ti, maxb):
            if ti == 0:
                pb_tiles[sb] = []
                sumacc[sb] = p_aux.tile(
                    [P, SB], FP32R, name=f"sumacc{sb}", tag="sumacc"
                )
            s_t = sc_tiles[sb][ti]
            nc.gpsimd.tensor_sub(s_t[:], s_t[:], maxb[:])
            p_t = p_pb.tile([P, SB], BF16, name=f"pb{sb}_{ti}", tag="pb")
            nc.scalar.activation(p_t[:], s_t[:], AF.Exp, bias=0.0, scale=1.0)
            if ti == 0:
                nc.vector.tensor_copy(sumacc[sb][:], p_t[:])
            else:
                nc.vector.tensor_add(sumacc[sb][:], sumacc[sb][:], p_t[:])
            pb_tiles[sb].append(p_t)

        def emit_sum_recip(sb):
            """rowsum over partitions via ones-matmul, then fast reciprocal."""
            ps_sum = p_psum.tile([1, SB], FP32, name=f"psum{sb}", tag="ps")
            nc.tensor.matmul(ps_sum[:], ones_col[:], sumacc[sb][:], start=True, stop=True)
            rrow = p_row.tile([1, SB], FP32, name=f"rrow{sb}", tag="rrow")
            with nc.allow_low_precision(reason="softmax denom reciprocal; fp22 ok"):
                nc.vector.reciprocal_approx_fast(rrow[:], ps_sum[:])
            return rrow

        def emit_recip_bcast(sb, rrow):
            ps_rb = p_psum.tile([P, SB], FP32, name=f"prb{sb}", tag="ps")
            nc.tensor.matmul(ps_rb[:], ones_row32[:], rrow[:], start=True, stop=True)
            recipb = p_aux.tile([P, SB], FP32, name=f"recipb{sb}", tag="recipb")
            nc.vector.tensor_copy(recipb[:], ps_rb[:])
            return recipb

        def mm4_copy_out(sb, ps_list, ots, et0):
            for i, ps_o in enumerate(ps_list):
                o_t = p_ot.tile([P, SB], OT_DT, name=f"ot{sb}_{et0 + i}", tag="ot")
                nc.vector.tensor_copy(o_t[:], ps_o[:])
                ots.append(o_t)

        def emit_mm5(sb, ots, recipb):
            s0 = sb * SB
            for ft in range(FT):
                ps_f = p_psum.tile([P, SB], FP32, name=f"psf{sb}_{ft}", tag="ps")
                for ec in range(DC):
                    nc.tensor.matmul(
                        ps_f[:], fcw_sl(ec, ft), ots[ec][:],
                        start=(ec == 0), stop=(ec == DC - 1),
                    )
                tmp = p_tmp.tile([P, SB], FP32, name=f"tmp{sb}_{ft}", tag="tmp")
                nc.vector.tensor_mul(tmp[:], ps_f[:], recipb[:])
                o_out = p_oo.tile([P, SB], FP32, name=f"oo{sb}_{ft}", tag="oo")
                nc.scalar.activation(
                    o_out[:], tmp[:], AF.Relu,
                    bias=fcb_cols[:, ft : ft + 1], scale=1.0,
                )
                nc.sync.dma_start(outt_d[ft * P : (ft + 1) * P, s0 : s0 + SB], o_out[:])

        # ---------- steady-state blocks ----------
        for sb in range(NSB):
            if sb + 2 < NSB:
                x2t_tiles[sb + 2] = issue_x2t(sb + 2)
            if sb + 1 < NSB:
                issue_x1tg((sb + 1) * NTG, 0)

            maxb = emit_max_reduce(sb)

            if sb + 1 < NSB:
                # PE chews MM2(sb+1) while GpSimd/Scalar/DVE run exp(sb)
                for ti in range(TC):
                    emit_exp_tile(sb, ti, maxb)
                    tg, sub = divmod(ti, SB // P)
                    if sub == 0 and tg + 1 < NTG:
                        issue_x1tg((sb + 1) * NTG + tg + 1, tg + 1)
                    emit_mm2_tile(sb + 1, ti)
                if sb + 2 < NSB:
                    emit_mm1(sb + 2)
                rrow = emit_sum_recip(sb)
                ots = []
                for et in range(DC):
                    ps_o = p_psum.tile([P, SB], FP32, name=f"pso{sb}_{et}", tag="ps")
                    for ti in range(TC):
                        nc.tensor.matmul(
                            ps_o[:], x1_sl(ti, et), pb_tiles[sb][ti][:],
                            start=(ti == 0), stop=(ti == TC - 1),
                        )
                    mm4_copy_out(sb, [ps_o], ots, et)
                recipb = emit_recip_bcast(sb, rrow)
                emit_mm5(sb, ots, recipb)
            else:
                # last block: full ti-outer MM4 in 8 held banks (both pools)
                mm4_ps = [
                    p_psum1.tile([P, SB], FP32, name=f"pso{sb}_{et}", tag="ps1")
                    for et in range(4)
                ] + [
                    p_psum.tile([P, SB], FP32, name=f"pso{sb}_{et}", tag="ps")
                    for et in range(4, DC)
                ]
                for ti in range(TC):
                    emit_exp_tile(sb, ti, maxb)
                    for et in range(DC):
                        nc.tensor.matmul(
                            mm4_ps[et][:], x1_sl(ti, et), pb_tiles[sb][ti][:],
                            start=(ti == 0), stop=(ti == TC - 1),
                        )
                ots = []
                mm4_copy_out(sb, mm4_ps, ots, 0)
                rrow = emit_sum_recip(sb)
                recipb = emit_recip_bcast(sb, rrow)
                emit_mm5(sb, ots, recipb)

    nc.compile()
    return nc


_NC_CACHE = None


def _get_nc():
    global _NC_CACHE
    if _NC_CACHE is None:
        _NC_CACHE = build_nc()
    return _NC_CACHE


def make_in_maps(x1, x2, U, bias, fc_w, fc_b):
    import ml_dtypes

    x1 = np.ascontiguousarray(np.asarray(x1, dtype=np.float32))
    x2 = np.ascontiguousarray(np.asarray(x2, dtype=np.float32))
    U = np.ascontiguousarray(np.asarray(U, dtype=np.float32))
    bias = np.asarray(bias, dtype=np.float32)
    fc_w = np.asarray(fc_w, dtype=np.float32)
    fc_b = np.asarray(fc_b, dtype=np.float32)
    # relaid residents (same for every core)
    ug = np.ascontiguousarray(
        U.reshape(DC, P, D).transpose(1, 0, 2).reshape(P, DC * D)
    )
    fcwg = np.ascontiguousarray(
        fc_w.T.reshape(DC, P, F).transpose(1, 0, 2).reshape(P, DC * F)
    ).astype(ml_dtypes.bfloat16)
    biasg = np.ascontiguousarray(bias.reshape(DC, P).T)
    fcbg = np.ascontiguousarray(fc_b.reshape(FT, P).T)
    in_maps = []
    for b in range(B):
        x1t = x1[b].T  # [D, S]
        x1tg = np.ascontiguousarray(
            x1t.reshape(DC, P, NTG, SB).transpose(1, 2, 0, 3).reshape(P, NTG * DC * SB)
        )
        x1g = np.ascontiguousarray(
            x1[b].reshape(TC, P, D).transpose(1, 0, 2).reshape(P, TC * D)
        ).astype(ml_dtypes.bfloat16)
        in_maps.append(
            {
                "x1g": x1g,
                "x1tg": x1tg,
                "x2t": np.ascontiguousarray(x2[b].T),
                "ug": ug,
                "fcwg": fcwg,
                "biasg": biasg,
                "fcbg": fcbg,
            }
        )
    return in_maps


def kernel(x1, x2, U, bias, fc_w, fc_b):
    from concourse.bass_utils import run_bass_kernel_spmd

    nc = _get_nc()
    in_maps = make_in_maps(x1, x2, U, bias, fc_w, fc_b)
    res = run_bass_kernel_spmd(nc, in_maps, core_ids=list(range(B)))
    out = np.stack([np.ascontiguousarray(r["outt"].T) for r in res.results])
    return out.astype(np.float32)


# revision 20
# speedup vs baseline: 1.5691x; 1.0029x over previous
"""BiaffineAttn Trainium2 kernel.

Math (per batch b):
    t    = x2 @ U + bias[None, :]      [S, D]   (bias folded: x2 U x1^T + 1 (x1 bias)^T
                                                 == [x2|1] [[U];[bias^T]] x1^T)
    attn = t @ x1^T
    p    = softmax(attn, axis=-1)
    out  = relu((p @ x1) @ fc_w^T + fc_b)       [S, F]

Sharding: data-parallel over batch B=8, one batch per NeuronCore.

Per-core pipeline, transposed orientation (softmax key dim t' on partitions),
software-pipelined across S-superblocks so the PE never idles during softmax.
The interleave window overlays three instruction streams per t'-tile:
  PE:     MM2(sb+1) tile (8 mm, 4-buf PSUM ring) + one MM1(sb+2) dc-step
          (4 mm into 4 held PSUM banks)  -> 12 mm/tile keeps PE the pacer
  Scalar: exp(sb, tile) + MM2 psum->SBUF score copy
  DVE:    running max + softmax-denominator accumulation
  GpSimd: score - maxb subtract (SBUF-only; Pool cannot touch PSUM)
Block 2 has no MM1 to interleave, so half of MM4(2) (et 0-3, ti-outer) fills
the window; block 3 interleaves the full ti-outer MM4(3) in 8 held banks.

rowsum via single ones-column matmul; reciprocal_approx_fast; relu+bias via
Scalar activation; [F,SB] stores per superblock.

All tensors are host-relaid so every resident loads in ONE big DMA (32KB
rows) and each x1t t'-group is a single [128, 4096] transfer (2-deep ring).

Host side: builds the relaid views and transposes the [F,S] per-core output
back to [S,F] when gathering (fp32 DMA transpose does not exist on TRN2).
"""

import os
import sys
from contextlib import ExitStack

import numpy as np

for _p in ("/opt/trn_rl_repo", os.path.expanduser("~/.axon_site/_ro/trn_rl_repo")):
    if os.path.isdir(_p) and _p not in sys.path:
        sys.path.insert(0, _p)

import concourse.bass as bass
import concourse.mybir as mybir
import concourse.tile as tile
from concourse import bacc

B = 8
S = 2048          # sequence length (both s and t')
D = 1024          # d_model
F = 512           # fc output dim
P = 128
SB = 512          # s superblock (moving free dim of every matmul)
NSB = S // SB     # 4
DC = D // P       # 8 contraction chunks of d / e
TC = S // P       # 16 t' tiles
NTG = NSB         # 4 t' groups of 4 tiles
FT = F // P       # 4
FP32 = mybir.dt.float32
FP32R = mybir.dt.float32r
BF16 = mybir.dt.bfloat16
AF = mybir.ActivationFunctionType
ALU = mybir.AluOpType
AX = mybir.AxisListType

OT_DT = BF16      # dtype of MM4 output tiles (MM5 moving operand)


def build_nc():
    nc = bacc.Bacc(
        "TRN2",
        target_bir_lowering=False,
        debug=False,
        enable_asserts=False,
    )

    # host-relaid tensors: row p holds the p-th partition's data for every tile
    x1_d = nc.dram_tensor("x1g", [P, TC * D], BF16, kind="ExternalInput")
    x1t_d = nc.dram_tensor("x1tg", [P, NTG * DC * SB], FP32R, kind="ExternalInput")
    x2t_d = nc.dram_tensor("x2t", [D, S], FP32R, kind="ExternalInput")
    u_d = nc.dram_tensor("ug", [P, DC * D], FP32R, kind="ExternalInput")
    fcwt_d = nc.dram_tensor("fcwg", [P, DC * F], BF16, kind="ExternalInput")
    bias_d = nc.dram_tensor("biasg", [P, DC], FP32, kind="ExternalInput")
    fcb_d = nc.dram_tensor("fcbg", [P, FT], FP32, kind="ExternalInput")
    outt_d = nc.dram_tensor("outt", [F, S], FP32, kind="ExternalOutput")

    with tile.TileContext(nc) as tc, ExitStack() as ctx:
        # ---------- pools ----------
        p_u = ctx.enter_context(tc.tile_pool(name="ures", bufs=1))
        p_x1 = ctx.enter_context(tc.tile_pool(name="x1res", bufs=1))
        p_bc = ctx.enter_context(tc.tile_pool(name="biascols", bufs=1))
        p_fcb = ctx.enter_context(tc.tile_pool(name="fcbcols", bufs=1))
        p_fcw = ctx.enter_context(tc.tile_pool(name="fcwres", bufs=1))
        p_ones = ctx.enter_context(tc.tile_pool(name="ones", bufs=1))
        p_psum = ctx.enter_context(tc.tile_pool(name="psum", bufs=4, space="PSUM"))
        p_psum1 = ctx.enter_context(tc.tile_pool(name="psum1", bufs=4, space="PSUM"))
        p_x2t = ctx.enter_context(tc.tile_pool(name="x2ts", bufs=5))
        p_x1tc = ctx.enter_context(tc.tile_pool(name="x1tgs", bufs=2))
        p_tt = ctx.enter_context(tc.tile_pool(name="tts", bufs=9))
        p_sc = ctx.enter_context(tc.tile_pool(name="scores", bufs=TC))
        p_pb = ctx.enter_context(tc.tile_pool(name="pbf", bufs=TC))
        p_ot = ctx.enter_context(tc.tile_pool(name="ots", bufs=DC))
        p_aux = ctx.enter_context(tc.tile_pool(name="aux", bufs=1))
        p_row = ctx.enter_context(tc.tile_pool(name="rows", bufs=1))
        p_tmp = ctx.enter_context(tc.tile_pool(name="tmps", bufs=1))
        p_oo = ctx.enter_context(tc.tile_pool(name="oos", bufs=1))

        # ---------- prologue DMAs: MM1(0) critical path first ----------
        u_big = p_u.tile([P, DC * D], FP32R, name="ug", tag="ur")
        for q in range(4):
            nc.sync.dma_start(
                u_big[:, q * 2 * D : (q + 1) * 2 * D],
                u_d[:, q * 2 * D : (q + 1) * 2 * D],
            )

        def u_sl(dc, et):
            return u_big[:, dc * D + et * P : dc * D + (et + 1) * P]

        x2t_tiles = {0: []}
        for dc in range(DC):
            x2_t = p_x2t.tile([P, SB], FP32R, name=f"x2t0_{dc}", tag="x2t")
            nc.sync.dma_start(x2_t[:], x2t_d[dc * P : (dc + 1) * P, 0:SB])
            x2t_tiles[0].append(x2_t)

        # x1t t'-group streaming (MM2 stationaries): one DMA per group, ring 2
        x1tg = {}

        def issue_x1tg(key, tg):
            t = p_x1tc.tile([P, DC * SB], FP32R, name=f"x1tg{key}", tag="x1tg")
            nc.sync.dma_start(t[:], x1t_d[:, tg * DC * SB : (tg + 1) * DC * SB])
            x1tg[key] = t

        issue_x1tg(0, 0)
        bias_cols = p_bc.tile([P, DC], FP32, name="bc", tag="bc")
        nc.sync.dma_start(bias_cols[:], bias_d[:, :])
        issue_x1tg(1, 1)

        x1_big = p_x1.tile([P, TC * D], BF16, name="x1g", tag="x1r")
        nc.sync.dma_start(x1_big[:], x1_d[:, :])

        def x1_sl(ti, et):
            return x1_big[:, ti * D + et * P : ti * D + (et + 1) * P]

        fcw_big = p_fcw.tile([P, DC * F], BF16, name="fcwg", tag="fcw")
        nc.sync.dma_start(fcw_big[:], fcwt_d[:, :])

        def fcw_sl(ec, ft):
            return fcw_big[:, ec * F + ft * P : ec * F + (ft + 1) * P]

        fcb_cols = p_fcb.tile([P, FT], FP32, name="fcb", tag="fcb")
        nc.sync.dma_start(fcb_cols[:], fcb_d[:, :])

        identity32 = p_ones.tile([P, P], FP32, name="ident32", tag="ident32")
        nc.gpsimd.memset(identity32[:], 0.0)
        nc.gpsimd.affine_select(
            out=identity32[:], in_=identity32[:], compare_op=ALU.not_equal,
            fill=1.0, base=0, pattern=[[-1, P]], channel_multiplier=1,
        )
        identity = p_ones.tile([P, P], FP32R, name="ident", tag="ident")
        nc.scalar.activation(identity[:], identity32[:], AF.Identity, bias=0.0, scale=1.0)
        ones_row = p_ones.tile([1, P], FP32R, name="ones_row", tag="ones_row")
        nc.scalar.activation(ones_row[:], identity[0:1, :], AF.Identity, bias=1.0, scale=0.0)
        ones_colb = p_ones.tile([P, 1], BF16, name="ones_colb", tag="ones_colb")
        nc.scalar.activation(ones_colb[:], identity[:, 0:1], AF.Identity, bias=1.0, scale=0.0)
        ones_row32 = p_ones.tile([1, P], FP32, name="ones_row32", tag="ones_row32")
        nc.scalar.activation(ones_row32[:], identity[0:1, :], AF.Identity, bias=1.0, scale=0.0)

        def issue_x2t(sb):
            tiles = []
            for dc in range(DC):
                x2_t = p_x2t.tile([P, SB], FP32R, name=f"x2t{sb}_{dc}", tag="x2t")
                nc.sync.dma_start(
                    x2_t[:], x2t_d[dc * P : (dc + 1) * P, sb * SB : (sb + 1) * SB]
                )
                tiles.append(x2_t)
            return tiles

        x2t_tiles[1] = issue_x2t(1)

        # ---------- MM1: ttT = (x2 @ U)^T + bias, 8 held banks (both pools) ----------
        tt = {}

        def emit_mm1(sb):
            tt[sb] = []
            ps = [
                p_psum1.tile([P, SB], FP32, name=f"ps1_{sb}_{et}", tag="ps1")
                for et in range(4)
            ] + [
                p_psum.tile([P, SB], FP32, name=f"ps1b_{sb}_{et}", tag="ps")
                for et in range(4, DC)
            ]
            for dc in range(DC):
                for et in range(DC):
                    nc.tensor.matmul(
                        ps[et][:], u_sl(dc, et), x2t_tiles[sb][dc][:],
                        start=(dc == 0), stop=(dc == DC - 1),
                    )
            for et in range(DC):
                t_t = p_tt.tile([P, SB], FP32R, name=f"tt{sb}_{et}", tag="tt")
                nc.scalar.activation(
                    t_t[:], ps[et][:], AF.Identity,
                    bias=bias_cols[:, et : et + 1], scale=1.0,
                )
                tt[sb].append(t_t)

        emit_mm1(0)

        # ---------- MM2 per-tile emitter: scoresT tile + running max ----------
        sc_tiles = {}
        maxacc = {}
        sumacc = {}
        pb_tiles = {}

        def emit_mm2_tile(sb, ti):
            tg, sub = divmod(ti, SB // P)
            if ti == 0:
                sc_tiles[sb] = []
                maxacc[sb] = p_aux.tile(
                    [P, SB], FP32R, name=f"maxacc{sb}", tag="maxacc"
                )
            ps_s = p_psum.tile([P, SB], FP32, name=f"pss{sb}_{ti}", tag="ps")
            grp = x1tg[sb * NTG + tg]
            for ec in range(DC):
                nc.tensor.matmul(
                    ps_s[:],
                    grp[:, ec * SB + sub * P : ec * SB + (sub + 1) * P],
                    tt[sb][ec][:],
                    start=(ec == 0), stop=(ec == DC - 1),
                )
            s_t = p_sc.tile([P, SB], FP32, name=f"sc{sb}_{ti}", tag="sc")
            nc.scalar.copy(s_t[:], ps_s[:])
            if ti == 0:
                nc.vector.tensor_copy(maxacc[sb][:], s_t[:])
            else:
                nc.vector.tensor_max(maxacc[sb][:], maxacc[sb][:], s_t[:])
            sc_tiles[sb].append(s_t)

        # prologue MM2(0) with group prefetch, then MM1(1)
        for ti in range(TC):
            tg, sub = divmod(ti, SB // P)
            if sub == 0 and tg + 2 < NTG:
                issue_x1tg(tg + 2, tg + 2)
            emit_mm2_tile(0, ti)
        emit_mm1(1)

        # ---------- softmax helpers ----------
        def emit_max_reduce(sb):
            """per-s max over partitions -> maxb [P,SB] broadcast tile."""
            mrow = p_row.tile([1, SB], FP32R, name=f"mrow{sb}", tag="mrow")
            trs, mcols = [], []
            for blk in range(SB // P):
                ps_tr = p_psum.tile([P, P], FP32R, name=f"ptr{sb}_{blk}", tag="ps")
                nc.tensor.transpose(
                    ps_tr[:], maxacc[sb][:, blk * P : (blk + 1) * P], identity[:]
                )
                trs.append(ps_tr)
            for blk in range(SB // P):
                mcol = p_row.tile([P, 1], FP32R, name=f"mcol{sb}_{blk}", tag=f"mcol{blk % 2}")
                nc.vector.reduce_max(mcol[:], trs[blk][:], axis=AX.X)
                mcols.append(mcol)
            ps_rrs = []
            for blk in range(SB // P):
                ps_rr = p_psum.tile([1, P], FP32R, name=f"prr{sb}_{blk}", tag="ps")
                nc.tensor.transpose(ps_rr[:], mcols[blk][:], identity[:])
                ps_rrs.append(ps_rr)
            for blk in range(SB // P):
                nc.vector.tensor_copy(mrow[:, blk * P : (blk + 1) * P], ps_rrs[blk][:])
            ps_mb = p_psum.tile([P, SB], FP32, name=f"pmb{sb}", tag="ps")
            nc.tensor.matmul(ps_mb[:], ones_row[:], mrow[:], start=True, stop=True)
            maxb = p_aux.tile([P, SB], FP32, name=f"maxb{sb}", tag="maxb")
            nc.vector.tensor_copy(maxb[:], ps_mb[:])
            return maxb

        def emit_exp_tile(sb, ti, maxb):
            if ti == 0:
                pb_tiles[sb] = []
            s_t = sc_tiles[sb][ti]
            nc.vector.tensor_sub(s_t[:], s_t[:], maxb[:])
            p_t = p_pb.tile([P, SB], BF16, name=f"pb{sb}_{ti}", tag="pb")
            nc.scalar.activation(p_t[:], s_t[:], AF.Exp, bias=0.0, scale=1.0)
            pb_tiles[sb].append(p_t)

        def emit_sum_mm(sb, ti, ps_sum):
            """accumulate softmax denominator on the PE: [1,SB] += 1^T @ pb."""
            nc.tensor.matmul(
                ps_sum[:], ones_colb[:], pb_tiles[sb][ti][:],
                start=(ti == 0), stop=(ti == TC - 1),
            )

        def emit_sum_recip(sb, ps_sum):
            rrow = p_row.tile([1, SB], FP32, name=f"rrow{sb}", tag="rrow")
            with nc.allow_low_precision(reason="softmax denom reciprocal; fp22 ok"):
                nc.vector.reciprocal_approx_fast(rrow[:], ps_sum[:])
            return rrow

        def emit_recip_bcast(sb, rrow):
            ps_rb = p_psum.tile([P, SB], FP32, name=f"prb{sb}", tag="ps")
            nc.tensor.matmul(ps_rb[:], ones_row32[:], rrow[:], start=True, stop=True)
            recipb = p_aux.tile([P, SB], FP32, name=f"recipb{sb}", tag="recipb")
            nc.vector.tensor_copy(recipb[:], ps_rb[:])
            return recipb

        def mm4_copy_out(sb, ps_list, ots, et0):
            for i, ps_o in enumerate(ps_list):
                o_t = p_ot.tile([P, SB], OT_DT, name=f"ot{sb}_{et0 + i}", tag="ot")
                nc.vector.tensor_copy(o_t[:], ps_o[:])
                ots.append(o_t)

        def emit_mm5(sb, ots, recipb):
            s0 = sb * SB
            for ft in range(FT):
                ps_f = p_psum.tile([P, SB], FP32, name=f"psf{sb}_{ft}", tag="ps")
                for ec in range(DC):
                    nc.tensor.matmul(
                        ps_f[:], fcw_sl(ec, ft), ots[ec][:],
                        start=(ec == 0), stop=(ec == DC - 1),
                    )
                tmp = p_tmp.tile([P, SB], FP32, name=f"tmp{sb}_{ft}", tag="tmp")
                nc.vector.tensor_mul(tmp[:], ps_f[:], recipb[:])
                o_out = p_oo.tile([P, SB], FP32, name=f"oo{sb}_{ft}", tag="oo")
                nc.scalar.activation(
                    o_out[:], tmp[:], AF.Relu,
                    bias=fcb_cols[:, ft : ft + 1], scale=1.0,
                )
                nc.sync.dma_start(outt_d[ft * P : (ft + 1) * P, s0 : s0 + SB], o_out[:])

        # ---------- steady-state blocks ----------
        for sb in range(NSB):
            if sb + 2 < NSB:
                x2t_tiles[sb + 2] = issue_x2t(sb + 2)
            if sb + 1 < NSB:
                issue_x1tg((sb + 1) * NTG, 0)

            maxb = emit_max_reduce(sb)

            if sb + 1 < NSB:
                # PE chews MM2(sb+1) + denom matmuls while Scalar/DVE run exp(sb)
                ps_sum = p_psum1.tile([1, SB], FP32, name=f"psum{sb}", tag="ps1")
                for ti in range(TC):
                    emit_exp_tile(sb, ti, maxb)
                    tg, sub = divmod(ti, SB // P)
                    if sub == 0 and tg + 1 < NTG:
                        issue_x1tg((sb + 1) * NTG + tg + 1, tg + 1)
                    emit_mm2_tile(sb + 1, ti)
                    emit_sum_mm(sb, ti, ps_sum)
                rrow = emit_sum_recip(sb, ps_sum)
                if sb + 2 < NSB:
                    emit_mm1(sb + 2)
                ots = []
                for et in range(DC):
                    ps_o = p_psum.tile([P, SB], FP32, name=f"pso{sb}_{et}", tag="ps")
                    for ti in range(TC):
                        nc.tensor.matmul(
                            ps_o[:], x1_sl(ti, et), pb_tiles[sb][ti][:],
                            start=(ti == 0), stop=(ti == TC - 1),
                        )
                    mm4_copy_out(sb, [ps_o], ots, et)
                recipb = emit_recip_bcast(sb, rrow)
                emit_mm5(sb, ots, recipb)
            else:
                # last block: ti-outer MM4 on 7 held banks + denom bank; et7 after
                mm4_ps = [
                    p_psum1.tile([P, SB], FP32, name=f"pso{sb}_{et}", tag="ps1")
                    for et in range(4)
                ] + [
                    p_psum.tile([P, SB], FP32, name=f"pso{sb}_{et}", tag="ps")
                    for et in range(4, DC - 1)
                ]
                ps_sum = p_psum.tile([1, SB], FP32, name=f"psum{sb}", tag="ps")
                for ti in range(TC):
                    emit_exp_tile(sb, ti, maxb)
                    for et in range(DC - 1):
                        nc.tensor.matmul(
                            mm4_ps[et][:], x1_sl(ti, et), pb_tiles[sb][ti][:],
                            start=(ti == 0), stop=(ti == TC - 1),
                        )
                    emit_sum_mm(sb, ti, ps_sum)
                ots = []
                mm4_copy_out(sb, mm4_ps, ots, 0)
                rrow = emit_sum_recip(sb, ps_sum)
                ps_o7 = p_psum.tile([P, SB], FP32, name=f"pso{sb}_7", tag="ps")
                for ti in range(TC):
                    nc.tensor.matmul(
                        ps_o7[:], x1_sl(ti, DC - 1), pb_tiles[sb][ti][:],
                        start=(ti == 0), stop=(ti == TC - 1),
                    )
                mm4_copy_out(sb, [ps_o7], ots, DC - 1)
                recipb = emit_recip_bcast(sb, rrow)
                emit_mm5(sb, ots, recipb)

    nc.compile()
    return nc


_NC_CACHE = None


def _get_nc():
    global _NC_CACHE
    if _NC_CACHE is None:
        _NC_CACHE = build_nc()
    return _NC_CACHE


def make_in_maps(x1, x2, U, bias, fc_w, fc_b):
    import ml_dtypes

    x1 = np.ascontiguousarray(np.asarray(x1, dtype=np.float32))
    x2 = np.ascontiguousarray(np.asarray(x2, dtype=np.float32))
    U = np.ascontiguousarray(np.asarray(U, dtype=np.float32))
    bias = np.asarray(bias, dtype=np.float32)
    fc_w = np.asarray(fc_w, dtype=np.float32)
    fc_b = np.asarray(fc_b, dtype=np.float32)
    # relaid residents (same for every core)
    ug = np.ascontiguousarray(
        U.reshape(DC, P, D).transpose(1, 0, 2).reshape(P, DC * D)
    )
    fcwg = np.ascontiguousarray(
        fc_w.T.reshape(DC, P, F).transpose(1, 0, 2).reshape(P, DC * F)
    ).astype(ml_dtypes.bfloat16)
    biasg = np.ascontiguousarray(bias.reshape(DC, P).T)
    fcbg = np.ascontiguousarray(fc_b.reshape(FT, P).T)
    in_maps = []
    for b in range(B):
        x1t = x1[b].T  # [D, S]
        x1tg = np.ascontiguousarray(
            x1t.reshape(DC, P, NTG, SB).transpose(1, 2, 0, 3).reshape(P, NTG * DC * SB)
        )
        x1g = np.ascontiguousarray(
            x1[b].reshape(TC, P, D).transpose(1, 0, 2).reshape(P, TC * D)
        ).astype(ml_dtypes.bfloat16)
        in_maps.append(
            {
                "x1g": x1g,
                "x1tg": x1tg,
                "x2t": np.ascontiguousarray(x2[b].T),
                "ug": ug,
                "fcwg": fcwg,
                "biasg": biasg,
                "fcbg": fcbg,
            }
        )
    return in_maps


def kernel(x1, x2, U, bias, fc_w, fc_b):
    from concourse.bass_utils import run_bass_kernel_spmd

    nc = _get_nc()
    in_maps = make_in_maps(x1, x2, U, bias, fc_w, fc_b)
    res = run_bass_kernel_spmd(nc, in_maps, core_ids=list(range(B)))
    out = np.stack([np.ascontiguousarray(r["outt"].T) for r in res.results])
    return out.astype(np.float32)
